# revision 3
# baseline (speedup 1.0000x reference)
"""BatchHardTripletLoss on 8 Trainium2 NeuronCores.

Strategy (data-parallel over anchor rows, samples pre-sorted by label):
  - host sorts samples by label (loss is permutation-invariant); core c owns
    anchor rows [c*512, (c+1)*512).
  - per-core column ROTATION puts the core's same-label bands at columns
    [0, ~600): each row's positives are a contiguous [lo_i, hi_i) band.
  - distances via fp8(e4m3) DoubleRow matmuls (2x PE rate):
    psum w[m, n] = colterm[n] - 2 e_m.e_n, where colterm is injected by a
    tiny K=4 fp8 matmul (3-term hi/lo/lolo fp8 split, scaled /32) and the
    gram by two K=256 DoubleRow passes (D=512 paired as [128, 2, *]).
  - mining on DVE with per-row band bounds as s0/s1 scalars:
      hp = band-max over a 384-wide per-m-tile window (op A)
      hn = min(band-excluded min over group0 (op B),
               plain min over group1 (tensor_reduce))
  - host: add row terms, sqrt, validity via label bincount, mean.
"""

import dataclasses

import numpy as np
import ml_dtypes

import concourse.bacc as bacc
import concourse.mybir as mybir
from concourse.bass_utils import run_bass_kernel_spmd
from concourse.tile import TileContext
from concourse import dve_ops as _dve_ops
from concourse.dve_spec import (
    AluOp, C0, C1, C2, MaxNeg, Spec, Src0, Idx, lower, select,
)
from concourse.dve_uop import DveOpSpec

FLT_MAX_NEG = np.float32(-3.4028234663852886e38)


def _register_op(name, spec):
    for op in _dve_ops.OPS:
        if op.name == name:
            return op
    op = _dve_ops.DveOp(name, spec, subdim=False, uops_sha={})
    _dve_ops.OPS.append(op)
    opcode = _dve_ops._CUSTOM_DVE_ROW_BASE + len(_dve_ops.OPS) - 1
    assert opcode < 0x20
    _dve_ops._SUB_OPCODE_FOR_NAME[name] = opcode
    _dve_ops.CUSTOM_DVE_SPECS[name] = spec
    shas = {}
    for ver in ("v3", "v4"):
        s = DveOpSpec(name=name, opcode=opcode, uops=lower(spec, ver=ver),
                      rd1_en=False)
        shas[ver] = s.sha(ver)
    op = dataclasses.replace(op, uops_sha=shas)
    _dve_ops.OPS[-1] = op
    return op


_IN_BAND = (Idx >= C0) & (Idx < C1)

# accum_out = max over j of (in_band ? in0 : -FLT_MAX)  -> hardest positive
BAND_MAX_OP = _register_op(
    "ANT_BAND_MAX",
    Spec(
        body=select(_IN_BAND, Src0, MaxNeg),
        accum=AluOp.MAX,
        accum_init=MaxNeg,
        reference=lambda in0, s0, s1, imm2: np.where(
            (np.arange(in0.shape[-1]) >= np.asarray(s0)[..., None])
            & (np.arange(in0.shape[-1]) < np.asarray(s1)[..., None]),
            in0, FLT_MAX_NEG,
        ).astype(np.float32),
    ),
)

# accum_out = min over j of (in_band ? imm2 : in0)  -> hardest negative part
BAND_MIN_OP = _register_op(
    "ANT_BANDX_MIN",
    Spec(
        body=select(_IN_BAND, C2, Src0),
        accum=AluOp.MIN,
        accum_init=C2,
        reference=lambda in0, s0, s1, imm2: np.where(
            (np.arange(in0.shape[-1]) >= np.asarray(s0)[..., None])
            & (np.arange(in0.shape[-1]) < np.asarray(s1)[..., None]),
            np.float32(imm2), in0,
        ).astype(np.float32),
    ),
)

B = 4096          # batch (anchors)
D = 512           # embedding dim
N_CORES = 8
ROWS = B // N_CORES      # 512 anchor rows per core
P = 128                  # partitions
MT = ROWS // P           # 4 m-tiles per core
NW = 512                 # psum bank width (fp32)
GW = 2048                # column group width (4 banks)
NG = B // GW             # 2 column groups
KT = D // P              # 4 contraction k-tiles (paired 2x for DoubleRow)

MARGIN = 0.5
EPS = 1e-6
BIG = 65536.0            # band-exclusion sentinel for the min ops
CTS = 32.0               # colterm fp8 split scale
AW = 384                 # band-max window width per m-tile
WLO = [0, 64, 192, 320]  # band-max window start per m-tile (t*128-64, clamped)

_nc_cache = {}


def _build(reps=1):
    nc = bacc.Bacc("TRN2", target_bir_lowering=False)
    fp16 = mybir.dt.float16
    fp8 = mybir.dt.float8e4
    f32 = mybir.dt.float32
    DR = mybir.MatmulPerfMode.DoubleRow

    et = nc.dram_tensor("et", [D, B], fp8, kind="ExternalInput")
    eblk = nc.dram_tensor("eblk", [D, ROWS], fp8, kind="ExternalInput")
    cl = nc.dram_tensor("cl", [2, 2 * P], fp8, kind="ExternalInput")
    cr = nc.dram_tensor("cr", [2, 2 * B], fp8, kind="ExternalInput")
    bnd = nc.dram_tensor("bnd", [P, 4 * MT], f32, kind="ExternalInput")
    outd = nc.dram_tensor("out", [reps, 2 * MT, P], f32, kind="ExternalOutput")

    with TileContext(nc) as tc:
        with (
            tc.tile_pool(name="etp", bufs=1) as etp,
            tc.tile_pool(name="ebp", bufs=1) as ebp,
            tc.tile_pool(name="wp", bufs=2) as wp,
            tc.tile_pool(name="accp", bufs=MT) as accp,
            tc.tile_pool(name="psp", bufs=2, space="PSUM") as psp,
        ):
            # --- PE warmup: dense tiny matmuls while input DMAs run -------
            warm = etp.tile([P, 64], fp16, tag="warm")
            nc.vector.memset(warm, 0.0)
            wps = psp.tile([P, GW], f32, tag="ps", name="wps")
            for _ in range(40):
                nc.tensor.matmul(wps[:64, 0:64], warm[:, 0:64], warm[:, 0:64],
                                 start=True, stop=True)

            # --- input DMAs, critical-path first --------------------------
            cl_sb = etp.tile([2, 2 * P], fp8, tag="cl")
            nc.gpsimd.dma_start(out=cl_sb, in_=cl[:, :])
            cr_sb = etp.tile([2, 2 * B], fp8, tag="cr")
            nc.gpsimd.dma_start(out=cr_sb, in_=cr[:, :])
            bnd_sb = etp.tile([P, 4 * MT], f32, tag="bnd")
            nc.gpsimd.dma_start(out=bnd_sb, in_=bnd[:, :])
            eb_all = ebp.tile([P, KT * ROWS], fp8, tag="eb", name="eb_all")
            et_all = etp.tile([P, KT * B], fp8, tag="et", name="et_all")
            eb4 = eb_all.rearrange("p (k n) -> p k n", k=KT)
            et4 = et_all.rearrange("p (k n) -> p k n", k=KT)
            cr2 = cr_sb.rearrange("p (q n) -> p q n", q=2)
            cl2 = cl_sb.rearrange("p (q n) -> p q n", q=2)
            ebd4 = eblk.rearrange("(k p) n -> p k n", p=P)
            etd4 = et.rearrange("(k p) n -> p k n", p=P)
            nc.gpsimd.dma_start(out=eb4, in_=ebd4)
            nc.gpsimd.dma_start(out=et4[:, :, 0:GW], in_=etd4[:, :, 0:GW])
            nc.gpsimd.dma_start(out=et4[:, :, GW:B], in_=etd4[:, :, GW:B])
            ident = etp.tile([P, P], f32, tag="ident")
            from concourse.masks import make_identity
            make_identity(nc, ident)

            for r in range(reps):
                out_sb = accp.tile([P, 2 * MT], f32, tag="osb", name="osb")
                hnp = accp.tile([P, 2 * MT], f32, tag="hnp", name="hnp")
                for t in range(MT):
                    ms = slice(t * P, (t + 1) * P)
                    for g in range(NG):
                        ps = psp.tile([P, GW], f32, tag="ps", name="ps")
                        for j in range(GW // NW):
                            cs = slice(g * GW + j * NW, g * GW + (j + 1) * NW)
                            js = slice(j * NW, (j + 1) * NW)
                            # colterm injection (start=True clears bank)
                            nc.tensor.matmul(
                                ps[:, js], cl2, cr2[:, :, cs],
                                start=True, stop=False, perf_mode=DR,
                            )
                            # gram: w -= 2 e_m.e_n  (two K=256 fp8 passes)
                            for u in range(2):
                                nc.tensor.matmul(
                                    ps[:, js],
                                    eb4[:, 2 * u:2 * u + 2, ms],
                                    et4[:, 2 * u:2 * u + 2, cs],
                                    start=False, stop=(u == 1), perf_mode=DR,
                                )
                        if g == 0:
                            # hardest positive: band max on the window
                            scr = wp.tile([P, GW], f32, tag="scr", name="scr")
                            nc.vector._custom_dve(
                                BAND_MAX_OP,
                                out=scr[:, 0:AW],
                                in0=ps[:, WLO[t]:WLO[t] + AW],
                                s0=bnd_sb[:, t:t + 1],
                                s1=bnd_sb[:, MT + t:MT + t + 1],
                                accum_out=out_sb[:, t:t + 1],
                            )
                            # hardest negative, group 0: band-excluded min
                            nc.vector._custom_dve(
                                BAND_MIN_OP,
                                out=scr[:, 0:GW],
                                in0=ps,
                                s0=bnd_sb[:, 2 * MT + t:2 * MT + t + 1],
                                s1=bnd_sb[:, 3 * MT + t:3 * MT + t + 1],
                                imm2=BIG,
                                accum_out=hnp[:, 2 * t:2 * t + 1],
                            )
                        else:
                            # hardest negative, group 1: plain min
                            nc.vector.tensor_reduce(
                                hnp[:, 2 * t + 1:2 * t + 2], ps,
                                mybir.AxisListType.X, mybir.AluOpType.min,
                            )
                # combine group minima: [P, MT, 2] -> [P, MT]
                hnp2 = hnp.rearrange("p (t k) -> p t k", k=2)
                nc.vector.tensor_reduce(
                    out_sb[:, MT:2 * MT], hnp2,
                    mybir.AxisListType.X, mybir.AluOpType.min,
                )
                # pack outputs: [128, 8] -> [8, 128] via PE transpose, one DMA
                tr = psp.tile([P, GW], f32, tag="ps", name="tr")
                nc.tensor.transpose(tr[0:2 * MT, 0:P], out_sb, ident)
                out_tr = accp.tile([P, P], f32, tag="otr", name="otr")
                nc.vector.tensor_copy(out_tr[0:2 * MT, :], tr[0:2 * MT, 0:P])
                nc.sync.dma_start(out=outd[r], in_=out_tr[0:2 * MT, :])
    nc.compile()
    return nc


def _get_nc(reps=1):
    if reps not in _nc_cache:
        _nc_cache[reps] = _build(reps)
    return _nc_cache[reps]


def _prepare_inputs(embeddings, labels):
    f8 = ml_dtypes.float8_e4m3
    Ef = np.ascontiguousarray(np.asarray(embeddings, dtype=np.float32))
    lab = np.asarray(labels).astype(np.int64)
    perm = np.argsort(lab, kind="stable")
    Ef = Ef[perm]
    labp = lab[perm]

    sq = np.sum(Ef * Ef, axis=1, dtype=np.float32)          # [B]
    s = np.sum(Ef, axis=1, dtype=np.float32)                # [B]
    rowterm = (sq + 2.0 * EPS * s + D * EPS * EPS).astype(np.float32)
    colterm = (sq - 2.0 * EPS * s).astype(np.float32)

    # fp8 embeddings, scaled by sqrt(2) so gram = 2 e.e
    et8 = np.ascontiguousarray(
        (Ef * np.float32(np.sqrt(2.0))).astype(f8).T)          # [D, B]
    en8 = np.ascontiguousarray(
        (Ef * np.float32(-np.sqrt(2.0))).astype(f8).T)         # [D, B]

    # colterm 3-term fp8 split (scaled /CTS)
    ct = (colterm / np.float32(CTS)).astype(np.float32)
    hi = ct.astype(f8)
    lo = (ct - hi.astype(np.float32)).astype(f8)
    lolo = (ct - hi.astype(np.float32) - lo.astype(np.float32)).astype(f8)

    cl_a = np.zeros((2, 2 * P), dtype=f8)
    cl_a[0, :] = f8(CTS)          # slots (0,0)=hi, (1,0)=lo, (0,1)=lolo
    cl_a[1, 0:P] = f8(CTS)

    seg_start = np.searchsorted(labp, labp, side="left")
    seg_end = np.searchsorted(labp, labp, side="right")

    in_maps = []
    for c in range(N_CORES):
        r0, r1 = c * ROWS, (c + 1) * ROWS
        w0 = int(seg_start[r0])
        lo_b = (seg_start[r0:r1] - w0).astype(np.float32)
        hi_b = (seg_end[r0:r1] - w0).astype(np.float32)
        assert hi_b.max() <= GW - 1, hi_b.max()
        colperm = (np.arange(B) + w0) % B

        bnd_a = np.zeros((P, 4 * MT), dtype=np.float32)
        for t in range(MT):
            tl = lo_b[t * P:(t + 1) * P]
            th = hi_b[t * P:(t + 1) * P]
            assert tl.min() >= WLO[t] and th.max() <= WLO[t] + AW, (
                c, t, tl.min(), th.max())
            bnd_a[:, t] = tl - WLO[t]
            bnd_a[:, MT + t] = th - WLO[t]
            bnd_a[:, 2 * MT + t] = tl
            bnd_a[:, 3 * MT + t] = th

        cr_a = np.zeros((2, 2 * B), dtype=f8)
        cr_a[0, 0:B] = hi[colperm]
        cr_a[1, 0:B] = lo[colperm]
        cr_a[0, B:2 * B] = lolo[colperm]

        in_maps.append({
            "et": np.ascontiguousarray(et8[:, colperm]),
            "eblk": np.ascontiguousarray(en8[:, r0:r1]),
            "cl": cl_a,
            "cr": cr_a,
            "bnd": bnd_a,
        })
    return in_maps, labp, rowterm


def _postprocess(results, labp, rowterm):
    hp_raw = np.concatenate([r["out"][0][:MT].reshape(-1) for r in results])
    hn_raw = np.concatenate([r["out"][0][MT:].reshape(-1) for r in results])
    hp2 = hp_raw + rowterm
    hn2 = hn_raw + rowterm
    hp = np.sqrt(np.maximum(hp2, 0.0, dtype=np.float32))
    hn = np.sqrt(np.maximum(hn2, 0.0, dtype=np.float32))

    cnt_lab = np.bincount(labp, minlength=1)
    n_same = cnt_lab[labp]
    valid = (n_same > 1) & (n_same < B)
    per = np.where(valid, np.maximum(hp - hn + np.float32(MARGIN), 0.0), 0.0)
    cnt = np.float32(valid.sum())
    if cnt > 0:
        loss = np.float32(per.sum(dtype=np.float32) / max(cnt, np.float32(1.0)))
    else:
        loss = np.float32(0.0)
    return np.asarray(loss, dtype=np.float32)


def _run(in_maps, reps=1, **kw):
    nc = _get_nc(reps)
    return run_bass_kernel_spmd(nc, in_maps, core_ids=list(range(N_CORES)), **kw)


def kernel(embeddings, labels):
    in_maps, labp, rowterm = _prepare_inputs(embeddings, labels)
    res = _run(in_maps)
    return _postprocess(res.results, labp, rowterm)


# revision 6
# speedup vs baseline: 1.3435x; 1.3435x over previous
"""BatchHardTripletLoss on 8 Trainium2 NeuronCores.

Strategy (data-parallel over anchor rows, samples pre-sorted by label):
  - host sorts samples by label (loss is permutation-invariant); core c owns
    anchor rows [c*512, (c+1)*512).
  - per-core column ROTATION puts the core's same-label bands at columns
    [0, ~600): each row's positives are a contiguous [lo_i, hi_i) band.
  - distances via fp8(e4m3) DoubleRow matmuls (2x PE rate):
    psum w[m, n] = colterm[n] - 2 e_m.e_n, where colterm is injected by a
    tiny K=4 fp8 matmul (3-term hi/lo/lolo fp8 split, scaled /32) and the
    gram by two K=256 DoubleRow passes (D=512 paired as [128, 2, *]).
  - mining on DVE with per-row band bounds as s0/s1 scalars:
      hp = band-max over a 384-wide per-m-tile window (op A)
      hn = min(band-excluded min over group0 (op B),
               plain min over group1 (tensor_reduce))
  - host: add row terms, sqrt, validity via label bincount, mean.
"""

import dataclasses

import numpy as np
import ml_dtypes

import concourse.bacc as bacc
import concourse.mybir as mybir
from concourse.bass_utils import run_bass_kernel_spmd
from concourse.tile import TileContext
from concourse import dve_ops as _dve_ops
from concourse.dve_spec import (
    AluOp, C0, C1, C2, MaxNeg, Spec, Src0, Idx, lower, select,
)
from concourse.dve_uop import DveOpSpec

FLT_MAX_NEG = np.float32(-3.4028234663852886e38)


def _register_op(name, spec):
    for op in _dve_ops.OPS:
        if op.name == name:
            return op
    op = _dve_ops.DveOp(name, spec, subdim=False, uops_sha={})
    _dve_ops.OPS.append(op)
    opcode = _dve_ops._CUSTOM_DVE_ROW_BASE + len(_dve_ops.OPS) - 1
    assert opcode < 0x20
    _dve_ops._SUB_OPCODE_FOR_NAME[name] = opcode
    _dve_ops.CUSTOM_DVE_SPECS[name] = spec
    shas = {}
    for ver in ("v3", "v4"):
        s = DveOpSpec(name=name, opcode=opcode, uops=lower(spec, ver=ver),
                      rd1_en=False)
        shas[ver] = s.sha(ver)
    op = dataclasses.replace(op, uops_sha=shas)
    _dve_ops.OPS[-1] = op
    return op


_IN_BAND = (Idx >= C0) & (Idx < C1)

# accum_out = max over j of (in_band ? in0 : -FLT_MAX)  -> hardest positive
BAND_MAX_OP = _register_op(
    "ANT_BAND_MAX",
    Spec(
        body=select(_IN_BAND, Src0, MaxNeg),
        accum=AluOp.MAX,
        accum_init=MaxNeg,
        reference=lambda in0, s0, s1, imm2: np.where(
            (np.arange(in0.shape[-1]) >= np.asarray(s0)[..., None])
            & (np.arange(in0.shape[-1]) < np.asarray(s1)[..., None]),
            in0, FLT_MAX_NEG,
        ).astype(np.float32),
    ),
)

# accum_out = min over j of (in_band ? imm2 : in0)  -> hardest negative part
BAND_MIN_OP = _register_op(
    "ANT_BANDX_MIN",
    Spec(
        body=select(_IN_BAND, C2, Src0),
        accum=AluOp.MIN,
        accum_init=C2,
        reference=lambda in0, s0, s1, imm2: np.where(
            (np.arange(in0.shape[-1]) >= np.asarray(s0)[..., None])
            & (np.arange(in0.shape[-1]) < np.asarray(s1)[..., None]),
            np.float32(imm2), in0,
        ).astype(np.float32),
    ),
)

B = 4096          # batch (anchors)
D = 512           # embedding dim
N_CORES = 8
ROWS = B // N_CORES      # 512 anchor rows per core
P = 128                  # partitions
MT = ROWS // P           # 4 m-tiles per core
NW = 512                 # psum bank width (fp32)
GW = 2048                # column group width (4 banks)
NG = B // GW             # 2 column groups
KT = D // P              # 4 contraction k-tiles (paired 2x for DoubleRow)

MARGIN = 0.5
EPS = 1e-6
BIG = 65536.0            # band-exclusion sentinel for the min ops
CTS = 32.0               # colterm fp8 split scale
AW = 384                 # band-max window width per m-tile
WLO = [0, 64, 192, 320]  # band-max window start per m-tile (t*128-64, clamped)

_nc_cache = {}


def _build(reps=1):
    nc = bacc.Bacc("TRN2", target_bir_lowering=False)
    fp16 = mybir.dt.float16
    fp8 = mybir.dt.float8e4
    f32 = mybir.dt.float32
    DR = mybir.MatmulPerfMode.DoubleRow

    et = nc.dram_tensor("et", [D, B], fp8, kind="ExternalInput")
    eblk = nc.dram_tensor("eblk", [D, ROWS], fp8, kind="ExternalInput")
    cl = nc.dram_tensor("cl", [2, 2 * P], fp8, kind="ExternalInput")
    cr = nc.dram_tensor("cr", [2, 2 * B], fp8, kind="ExternalInput")
    bnd = nc.dram_tensor("bnd", [P, 4 * MT], f32, kind="ExternalInput")
    outd = nc.dram_tensor("out", [reps, 2 * MT, P], f32, kind="ExternalOutput")

    with TileContext(nc) as tc:
        with (
            tc.tile_pool(name="etp", bufs=1) as etp,
            tc.tile_pool(name="ebp", bufs=1) as ebp,
            tc.tile_pool(name="wp", bufs=2) as wp,
            tc.tile_pool(name="accp", bufs=MT) as accp,
            tc.tile_pool(name="psp", bufs=2, space="PSUM") as psp,
        ):
            # --- PE warmup: dense tiny matmuls while input DMAs run -------
            warm = etp.tile([P, 64], fp16, tag="warm")
            nc.vector.memset(warm, 0.0)
            wps = psp.tile([P, GW], f32, tag="ps", name="wps")
            for _ in range(96):
                nc.tensor.matmul(wps[:64, 0:64], warm[:, 0:64], warm[:, 0:64],
                                 start=True, stop=True)

            # --- input DMAs, critical-path first --------------------------
            cl_sb = etp.tile([2, 2 * P], fp8, tag="cl")
            nc.gpsimd.dma_start(out=cl_sb, in_=cl[:, :])
            cr_sb = etp.tile([2, 2 * B], fp8, tag="cr")
            nc.gpsimd.dma_start(out=cr_sb, in_=cr[:, :])
            bnd_sb = etp.tile([P, 4 * MT], f32, tag="bnd")
            nc.gpsimd.dma_start(out=bnd_sb, in_=bnd[:, :])
            eb_all = ebp.tile([P, KT * ROWS], fp8, tag="eb", name="eb_all")
            et_all = etp.tile([P, KT * B], fp8, tag="et", name="et_all")
            eb4 = eb_all.rearrange("p (k n) -> p k n", k=KT)
            et4 = et_all.rearrange("p (k n) -> p k n", k=KT)
            cr2 = cr_sb.rearrange("p (q n) -> p q n", q=2)
            cl2 = cl_sb.rearrange("p (q n) -> p q n", q=2)
            ebd4 = eblk.rearrange("(k p) n -> p k n", p=P)
            etd4 = et.rearrange("(k p) n -> p k n", p=P)
            nc.gpsimd.dma_start(out=eb4, in_=ebd4)
            nc.gpsimd.dma_start(out=et4[:, 0:2, 0:GW], in_=etd4[:, 0:2, 0:GW])
            nc.gpsimd.dma_start(out=et4[:, 2:4, 0:GW], in_=etd4[:, 2:4, 0:GW])
            nc.gpsimd.dma_start(out=et4[:, 0:2, GW:B], in_=etd4[:, 0:2, GW:B])
            nc.gpsimd.dma_start(out=et4[:, 2:4, GW:B], in_=etd4[:, 2:4, GW:B])
            ident = etp.tile([P, P], f32, tag="ident")
            from concourse.masks import make_identity
            make_identity(nc, ident)

            for r in range(reps):
                out_sb = accp.tile([P, 2 * MT], f32, tag="osb", name="osb")
                hnp = accp.tile([P, 2 * MT], f32, tag="hnp", name="hnp")
                for t in range(MT):
                    ms = slice(t * P, (t + 1) * P)
                    for g in range(NG):
                        ps = psp.tile([P, GW], f32, tag="ps", name="ps")
                        # k-major order: consecutive matmuls share the
                        # stationary operand (LDWEIGHTS reuse)
                        for j in range(GW // NW):
                            cs = slice(g * GW + j * NW, g * GW + (j + 1) * NW)
                            js = slice(j * NW, (j + 1) * NW)
                            # colterm injection (start=True clears bank)
                            nc.tensor.matmul(
                                ps[:, js], cl2, cr2[:, :, cs],
                                start=True, stop=False, perf_mode=DR,
                            )
                        # gram: w -= 2 e_m.e_n  (two K=256 fp8 passes)
                        for u in range(2):
                            for j in range(GW // NW):
                                cs = slice(g * GW + j * NW, g * GW + (j + 1) * NW)
                                js = slice(j * NW, (j + 1) * NW)
                                nc.tensor.matmul(
                                    ps[:, js],
                                    eb4[:, 2 * u:2 * u + 2, ms],
                                    et4[:, 2 * u:2 * u + 2, cs],
                                    start=False, stop=(u == 1), perf_mode=DR,
                                )
                        if g == 0:
                            # hardest positive: band max on the window
                            scr = wp.tile([P, GW], f32, tag="scr", name="scr")
                            nc.vector._custom_dve(
                                BAND_MAX_OP,
                                out=scr[:, 0:AW],
                                in0=ps[:, WLO[t]:WLO[t] + AW],
                                s0=bnd_sb[:, t:t + 1],
                                s1=bnd_sb[:, MT + t:MT + t + 1],
                                accum_out=out_sb[:, t:t + 1],
                            )
                            # hardest negative, group 0: band-excluded min
                            nc.vector._custom_dve(
                                BAND_MIN_OP,
                                out=scr[:, 0:GW],
                                in0=ps,
                                s0=bnd_sb[:, 2 * MT + t:2 * MT + t + 1],
                                s1=bnd_sb[:, 3 * MT + t:3 * MT + t + 1],
                                imm2=BIG,
                                accum_out=hnp[:, 2 * t:2 * t + 1],
                            )
                        else:
                            # hardest negative, group 1: plain min
                            nc.vector.tensor_reduce(
                                hnp[:, 2 * t + 1:2 * t + 2], ps,
                                mybir.AxisListType.X, mybir.AluOpType.min,
                            )
                # combine group minima: [P, MT, 2] -> [P, MT]
                hnp2 = hnp.rearrange("p (t k) -> p t k", k=2)
                nc.vector.tensor_reduce(
                    out_sb[:, MT:2 * MT], hnp2,
                    mybir.AxisListType.X, mybir.AluOpType.min,
                )
                # pack outputs: [128, 8] -> [8, 128] via PE transpose, one DMA
                tr = psp.tile([P, GW], f32, tag="ps", name="tr")
                nc.tensor.transpose(tr[0:2 * MT, 0:P], out_sb, ident)
                out_tr = accp.tile([P, P], f32, tag="otr", name="otr")
                nc.vector.tensor_copy(out_tr[0:2 * MT, :], tr[0:2 * MT, 0:P])
                nc.sync.dma_start(out=outd[r], in_=out_tr[0:2 * MT, :])
    nc.compile()
    return nc


def _get_nc(reps=1):
    if reps not in _nc_cache:
        _nc_cache[reps] = _build(reps)
    return _nc_cache[reps]


def _prepare_inputs(embeddings, labels):
    f8 = ml_dtypes.float8_e4m3
    Ef = np.ascontiguousarray(np.asarray(embeddings, dtype=np.float32))
    lab = np.asarray(labels).astype(np.int64)
    perm = np.argsort(lab, kind="stable")
    Ef = Ef[perm]
    labp = lab[perm]

    sq = np.sum(Ef * Ef, axis=1, dtype=np.float32)          # [B]
    s = np.sum(Ef, axis=1, dtype=np.float32)                # [B]
    rowterm = (sq + 2.0 * EPS * s + D * EPS * EPS).astype(np.float32)
    colterm = (sq - 2.0 * EPS * s).astype(np.float32)

    # fp8 embeddings, scaled by sqrt(2) so gram = 2 e.e
    et8 = np.ascontiguousarray(
        (Ef * np.float32(np.sqrt(2.0))).astype(f8).T)          # [D, B]
    en8 = np.ascontiguousarray(
        (Ef * np.float32(-np.sqrt(2.0))).astype(f8).T)         # [D, B]

    # colterm 3-term fp8 split (scaled /CTS)
    ct = (colterm / np.float32(CTS)).astype(np.float32)
    hi = ct.astype(f8)
    lo = (ct - hi.astype(np.float32)).astype(f8)
    lolo = (ct - hi.astype(np.float32) - lo.astype(np.float32)).astype(f8)

    cl_a = np.zeros((2, 2 * P), dtype=f8)
    cl_a[0, :] = f8(CTS)          # slots (0,0)=hi, (1,0)=lo, (0,1)=lolo
    cl_a[1, 0:P] = f8(CTS)

    seg_start = np.searchsorted(labp, labp, side="left")
    seg_end = np.searchsorted(labp, labp, side="right")

    in_maps = []
    for c in range(N_CORES):
        r0, r1 = c * ROWS, (c + 1) * ROWS
        w0 = int(seg_start[r0])
        lo_b = (seg_start[r0:r1] - w0).astype(np.float32)
        hi_b = (seg_end[r0:r1] - w0).astype(np.float32)
        assert hi_b.max() <= GW - 1, hi_b.max()
        colperm = (np.arange(B) + w0) % B

        bnd_a = np.zeros((P, 4 * MT), dtype=np.float32)
        for t in range(MT):
            tl = lo_b[t * P:(t + 1) * P]
            th = hi_b[t * P:(t + 1) * P]
            assert tl.min() >= WLO[t] and th.max() <= WLO[t] + AW, (
                c, t, tl.min(), th.max())
            bnd_a[:, t] = tl - WLO[t]
            bnd_a[:, MT + t] = th - WLO[t]
            bnd_a[:, 2 * MT + t] = tl
            bnd_a[:, 3 * MT + t] = th

        cr_a = np.zeros((2, 2 * B), dtype=f8)
        cr_a[0, 0:B] = hi[colperm]
        cr_a[1, 0:B] = lo[colperm]
        cr_a[0, B:2 * B] = lolo[colperm]

        in_maps.append({
            "et": np.ascontiguousarray(et8[:, colperm]),
            "eblk": np.ascontiguousarray(en8[:, r0:r1]),
            "cl": cl_a,
            "cr": cr_a,
            "bnd": bnd_a,
        })
    return in_maps, labp, rowterm


def _postprocess(results, labp, rowterm):
    hp_raw = np.concatenate([r["out"][0][:MT].reshape(-1) for r in results])
    hn_raw = np.concatenate([r["out"][0][MT:].reshape(-1) for r in results])
    hp2 = hp_raw + rowterm
    hn2 = hn_raw + rowterm
    hp = np.sqrt(np.maximum(hp2, 0.0, dtype=np.float32))
    hn = np.sqrt(np.maximum(hn2, 0.0, dtype=np.float32))

    cnt_lab = np.bincount(labp, minlength=1)
    n_same = cnt_lab[labp]
    valid = (n_same > 1) & (n_same < B)
    per = np.where(valid, np.maximum(hp - hn + np.float32(MARGIN), 0.0), 0.0)
    cnt = np.float32(valid.sum())
    if cnt > 0:
        loss = np.float32(per.sum(dtype=np.float32) / max(cnt, np.float32(1.0)))
    else:
        loss = np.float32(0.0)
    return np.asarray(loss, dtype=np.float32)


def _run(in_maps, reps=1, **kw):
    nc = _get_nc(reps)
    return run_bass_kernel_spmd(nc, in_maps, core_ids=list(range(N_CORES)), **kw)


def kernel(embeddings, labels):
    in_maps, labp, rowterm = _prepare_inputs(embeddings, labels)
    res = _run(in_maps)
    return _postprocess(res.results, labp, rowterm)


# revision 7
# speedup vs baseline: 1.4931x; 1.1113x over previous
"""BatchHardTripletLoss on 8 Trainium2 NeuronCores.

Strategy (data-parallel over anchor rows, samples pre-sorted by label):
  - host sorts samples by label (loss is permutation-invariant); core c owns
    anchor rows [c*512, (c+1)*512).
  - per-core column ROTATION puts the core's same-label bands at columns
    [0, ~600): each row's positives are a contiguous [lo_i, hi_i) band.
  - PE: psum = -2 e_m.e_n via fp8(e4m3) DoubleRow matmuls only (2x rate,
    D=512 paired as [128, 2, *]; k-major order so consecutive matmuls share
    the stationary operand and LDWEIGHTS pipelines).
  - DVE mining with host-precomputed fp16 "colterm + band mask" tiles on
    the second read port (in1):
      hp  = ADD_MAX over a 384-wide window, in1 = colterm - 8192*(~band)
      hn  = min(ADD_MIN group0, in1 = colterm + 8192*band;
                ADD_MIN group1, in1 = colterm)
  - host: add row terms, final min/transpose, sqrt, validity via label
    bincount, mean.
"""

import dataclasses

import numpy as np
import ml_dtypes

import concourse.bacc as bacc
import concourse.mybir as mybir
from concourse.bass_utils import run_bass_kernel_spmd
from concourse.tile import TileContext
from concourse import dve_ops as _dve_ops
from concourse.dve_spec import (
    AluOp, C2, MaxNeg, Spec, Src0, Src1, lower,
)
from concourse.dve_uop import DveOpSpec


def _register_op(name, spec):
    for op in _dve_ops.OPS:
        if op.name == name:
            return op
    op = _dve_ops.DveOp(name, spec, subdim=False, uops_sha={})
    _dve_ops.OPS.append(op)
    opcode = _dve_ops._CUSTOM_DVE_ROW_BASE + len(_dve_ops.OPS) - 1
    assert opcode < 0x20
    _dve_ops._SUB_OPCODE_FOR_NAME[name] = opcode
    _dve_ops.CUSTOM_DVE_SPECS[name] = spec
    shas = {}
    for ver in ("v3", "v4"):
        s = DveOpSpec(name=name, opcode=opcode, uops=lower(spec, ver=ver),
                      rd1_en=True)
        shas[ver] = s.sha(ver)
    op = dataclasses.replace(op, uops_sha=shas)
    _dve_ops.OPS[-1] = op
    return op


ADD_MIN_OP = _register_op(
    "ANT_ADD_MIN",
    Spec(
        body=Src0 + Src1,
        accum=AluOp.MIN,
        accum_init=C2,
        reference=lambda in0, in1, s0, s1, imm2: (in0 + in1).astype(np.float32),
    ),
)

ADD_MAX_OP = _register_op(
    "ANT_ADD_MAX",
    Spec(
        body=Src0 + Src1,
        accum=AluOp.MAX,
        accum_init=MaxNeg,
        reference=lambda in0, in1, s0, s1, imm2: (in0 + in1).astype(np.float32),
    ),
)

B = 4096          # batch (anchors)
D = 512           # embedding dim
N_CORES = 8
ROWS = B // N_CORES      # 512 anchor rows per core
P = 128                  # partitions
MT = ROWS // P           # 4 m-tiles per core
NW = 512                 # psum bank width (fp32)
GW = 2048                # column group width (4 banks)
NG = B // GW             # 2 column groups
KT = D // P              # 4 contraction k-tiles (paired 2x for DoubleRow)

MARGIN = 0.5
EPS = 1e-6
BIG = 65536.0            # accum_init for the min ops
PEN = 8192.0             # band bump baked into the fp16 in1 tiles
AW = 384                 # band-max window width per m-tile
WLO = [0, 64, 192, 320]  # band-max window start per m-tile (t*128-64, clamped)

_nc_cache = {}


def _build(reps=1):
    nc = bacc.Bacc("TRN2", target_bir_lowering=False)
    fp16 = mybir.dt.float16
    fp8 = mybir.dt.float8e4
    f32 = mybir.dt.float32
    DR = mybir.MatmulPerfMode.DoubleRow

    et = nc.dram_tensor("et", [D, B], fp8, kind="ExternalInput")
    eblk = nc.dram_tensor("eblk", [D, ROWS], fp8, kind="ExternalInput")
    ctmin = nc.dram_tensor("ctmin", [P, MT * GW], fp16, kind="ExternalInput")
    ctmax = nc.dram_tensor("ctmax", [P, MT * AW], fp16, kind="ExternalInput")
    ctg1 = nc.dram_tensor("ctg1", [P, GW], fp16, kind="ExternalInput")
    outd = nc.dram_tensor("out", [reps, P, 3 * MT], f32, kind="ExternalOutput")

    with TileContext(nc) as tc:
        with (
            tc.tile_pool(name="etp", bufs=1) as etp,
            tc.tile_pool(name="ebp", bufs=1) as ebp,
            tc.tile_pool(name="wp", bufs=2) as wp,
            tc.tile_pool(name="accp", bufs=2) as accp,
            tc.tile_pool(name="psp", bufs=2, space="PSUM") as psp,
        ):
            # --- PE warmup: dense tiny matmuls while input DMAs run -------
            warm = etp.tile([P, 64], fp16, tag="warm")
            nc.vector.memset(warm, 0.0)
            wps = psp.tile([P, GW], f32, tag="ps", name="wps")
            for _ in range(96):
                nc.tensor.matmul(wps[:64, 0:64], warm[:, 0:64], warm[:, 0:64],
                                 start=True, stop=True)

            # --- input DMAs on parallel queues, critical-path first -------
            eb_all = ebp.tile([P, KT * ROWS], fp8, tag="eb", name="eb_all")
            et_all = etp.tile([P, KT * B], fp8, tag="et", name="et_all")
            eb4 = eb_all.rearrange("p (k n) -> p k n", k=KT)
            et4 = et_all.rearrange("p (k n) -> p k n", k=KT)
            ebd4 = eblk.rearrange("(k p) n -> p k n", p=P)
            etd4 = et.rearrange("(k p) n -> p k n", p=P)
            nc.gpsimd.dma_start(out=eb4, in_=ebd4)
            nc.gpsimd.dma_start(out=et4[:, 0:2, 0:GW], in_=etd4[:, 0:2, 0:GW])
            nc.gpsimd.dma_start(out=et4[:, 2:4, 0:GW], in_=etd4[:, 2:4, 0:GW])
            nc.scalar.dma_start(out=et4[:, 0:2, GW:B], in_=etd4[:, 0:2, GW:B])
            nc.scalar.dma_start(out=et4[:, 2:4, GW:B], in_=etd4[:, 2:4, GW:B])
            ctmax_sb = etp.tile([P, MT * AW], fp16, tag="ctmax")
            nc.gpsimd.dma_start(out=ctmax_sb, in_=ctmax[:, :])
            ctmin_sb = etp.tile([P, MT * GW], fp16, tag="ctmin")
            for t in range(MT):
                ts_ = slice(t * GW, (t + 1) * GW)
                nc.sync.dma_start(out=ctmin_sb[:, ts_], in_=ctmin[:, ts_])
            ctg1_sb = etp.tile([P, GW], fp16, tag="ctg1")
            nc.sync.dma_start(out=ctg1_sb, in_=ctg1[:, :])

            for r in range(reps):
                osb = accp.tile([P, 3 * MT], f32, tag="osb", name="osb")
                for t in range(MT):
                    ms = slice(t * P, (t + 1) * P)
                    for g in range(NG):
                        ps = psp.tile([P, GW], f32, tag="ps", name="ps")
                        # k-major: consecutive matmuls share the stationary
                        # operand; gram: w = -2 e_m.e_n (two K=256 passes)
                        for u in range(2):
                            for j in range(GW // NW):
                                cs = slice(g * GW + j * NW,
                                           g * GW + (j + 1) * NW)
                                js = slice(j * NW, (j + 1) * NW)
                                nc.tensor.matmul(
                                    ps[:, js],
                                    eb4[:, 2 * u:2 * u + 2, ms],
                                    et4[:, 2 * u:2 * u + 2, cs],
                                    start=(u == 0), stop=(u == 1),
                                    perf_mode=DR,
                                )
                        scr = wp.tile([P, GW], f32, tag="scr", name="scr")
                        if g == 0:
                            # hardest positive: windowed max of w + ctmax
                            nc.vector._custom_dve(
                                ADD_MAX_OP,
                                out=scr[:, 0:AW],
                                in0=ps[:, WLO[t]:WLO[t] + AW],
                                in1=ctmax_sb[:, t * AW:(t + 1) * AW],
                                accum_out=osb[:, t:t + 1],
                            )
                            # hardest negative, group 0 (band pushed +PEN)
                            nc.vector._custom_dve(
                                ADD_MIN_OP,
                                out=scr,
                                in0=ps,
                                in1=ctmin_sb[:, t * GW:(t + 1) * GW],
                                imm2=BIG,
                                accum_out=osb[:, MT + t:MT + t + 1],
                            )
                        else:
                            # hardest negative, group 1 (no band columns)
                            nc.vector._custom_dve(
                                ADD_MIN_OP,
                                out=scr,
                                in0=ps,
                                in1=ctg1_sb,
                                imm2=BIG,
                                accum_out=osb[:, 2 * MT + t:2 * MT + t + 1],
                            )
                nc.sync.dma_start(out=outd[r], in_=osb)
    nc.compile()
    return nc


def _get_nc(reps=1):
    if reps not in _nc_cache:
        _nc_cache[reps] = _build(reps)
    return _nc_cache[reps]


def _prepare_inputs(embeddings, labels):
    f8 = ml_dtypes.float8_e4m3
    Ef = np.ascontiguousarray(np.asarray(embeddings, dtype=np.float32))
    lab = np.asarray(labels).astype(np.int64)
    perm = np.argsort(lab, kind="stable")
    Ef = Ef[perm]
    labp = lab[perm]

    sq = np.sum(Ef * Ef, axis=1, dtype=np.float32)          # [B]
    s = np.sum(Ef, axis=1, dtype=np.float32)                # [B]
    rowterm = (sq + 2.0 * EPS * s + D * EPS * EPS).astype(np.float32)
    colterm = (sq - 2.0 * EPS * s).astype(np.float32)

    # fp8 embeddings, scaled by sqrt(2) so gram = 2 e.e
    et8 = np.ascontiguousarray(
        (Ef * np.float32(np.sqrt(2.0))).astype(f8).T)          # [D, B]
    en8 = np.ascontiguousarray(
        (Ef * np.float32(-np.sqrt(2.0))).astype(f8).T)         # [D, B]

    seg_start = np.searchsorted(labp, labp, side="left")
    seg_end = np.searchsorted(labp, labp, side="right")

    jj = np.arange(GW)
    in_maps = []
    for c in range(N_CORES):
        r0, r1 = c * ROWS, (c + 1) * ROWS
        w0 = int(seg_start[r0])
        lo_b = (seg_start[r0:r1] - w0).astype(np.int64)
        hi_b = (seg_end[r0:r1] - w0).astype(np.int64)
        colperm = (np.arange(B) + w0) % B
        ctrot = colterm[colperm]

        ctmin_a = np.empty((P, MT * GW), dtype=np.float16)
        ctmax_a = np.empty((P, MT * AW), dtype=np.float16)
        for t in range(MT):
            tl = lo_b[t * P:(t + 1) * P][:, None]
            th = hi_b[t * P:(t + 1) * P][:, None]
            assert tl.min() >= WLO[t] and th.max() <= WLO[t] + AW, (
                c, t, tl.min(), th.max())
            band = (jj[None, :] >= tl) & (jj[None, :] < th)      # [P, GW]
            ctmin_a[:, t * GW:(t + 1) * GW] = (
                ctrot[None, 0:GW] + np.float32(PEN) * band)
            wnd = band[:, WLO[t]:WLO[t] + AW]
            ctmax_a[:, t * AW:(t + 1) * AW] = (
                ctrot[None, WLO[t]:WLO[t] + AW]
                - np.float32(PEN) * (~wnd))

        in_maps.append({
            "et": np.ascontiguousarray(et8[:, colperm]),
            "eblk": np.ascontiguousarray(en8[:, r0:r1]),
            "ctmin": ctmin_a,
            "ctmax": ctmax_a,
            "ctg1": np.ascontiguousarray(
                np.broadcast_to(ctrot[None, GW:B], (P, GW))
            ).astype(np.float16),
        })
    return in_maps, labp, rowterm


def _postprocess(results, labp, rowterm):
    # out[0]: [P, 3*MT]: hp in cols [0:MT], hn_g0 [MT:2MT], hn_g1 [2MT:3MT]
    hp_l, hn_l = [], []
    for r in results:
        o = r["out"][0]                                   # [P, 3*MT]
        hp_l.append(o[:, 0:MT].T.reshape(-1))
        hn_l.append(np.minimum(o[:, MT:2 * MT], o[:, 2 * MT:3 * MT])
                    .T.reshape(-1))
    hp_raw = np.concatenate(hp_l)
    hn_raw = np.concatenate(hn_l)
    hp2 = hp_raw + rowterm
    hn2 = hn_raw + rowterm
    hp = np.sqrt(np.maximum(hp2, 0.0, dtype=np.float32))
    hn = np.sqrt(np.maximum(hn2, 0.0, dtype=np.float32))

    cnt_lab = np.bincount(labp, minlength=1)
    n_same = cnt_lab[labp]
    valid = (n_same > 1) & (n_same < B)
    per = np.where(valid, np.maximum(hp - hn + np.float32(MARGIN), 0.0), 0.0)
    cnt = np.float32(valid.sum())
    if cnt > 0:
        loss = np.float32(per.sum(dtype=np.float32) / max(cnt, np.float32(1.0)))
    else:
        loss = np.float32(0.0)
    return np.asarray(loss, dtype=np.float32)


def _run(in_maps, reps=1, **kw):
    nc = _get_nc(reps)
    return run_bass_kernel_spmd(nc, in_maps, core_ids=list(range(N_CORES)), **kw)


def kernel(embeddings, labels):
    in_maps, labp, rowterm = _prepare_inputs(embeddings, labels)
    res = _run(in_maps)
    return _postprocess(res.results, labp, rowterm)


# revision 8
# speedup vs baseline: 1.5012x; 1.0055x over previous
"""BatchHardTripletLoss on 8 Trainium2 NeuronCores.

Strategy (data-parallel over anchor rows, samples pre-sorted by label):
  - host sorts samples by label (loss is permutation-invariant); core c owns
    anchor rows [c*512, (c+1)*512).
  - per-core column ROTATION puts the core's same-label bands at columns
    [0, ~600): each row's positives are a contiguous [lo_i, hi_i) band.
  - PE: psum = -2 e_m.e_n via fp8(e4m3) DoubleRow matmuls only (2x rate,
    D=512 paired as [128, 2, *]; k-major order so consecutive matmuls share
    the stationary operand and LDWEIGHTS pipelines).
  - DVE mining with host-precomputed fp16 "colterm + 8192*band" tiles on
    the second read port (in1); the band only lives in columns [0, BW) so
    the per-m-tile banded tile is narrow and the rest shares plain colterm:
      hp   = max(w + ctb + (-8192)) over a 384-wide window   (ADD_ADD_MAX)
      hn   = min over three ADD_MIN spans:
             [0:BW)+ctb   [BW:2048)+ctr   group1+ctg1
  - host: min of hn parts, add row terms, sqrt, validity via bincount, mean.
"""

import dataclasses

import numpy as np
import ml_dtypes

import concourse.bacc as bacc
import concourse.mybir as mybir
from concourse.bass_utils import run_bass_kernel_spmd
from concourse.tile import TileContext
from concourse import dve_ops as _dve_ops
from concourse.dve_spec import (
    AluOp, C2, MaxNeg, Spec, Src0, Src1, lower,
)
from concourse.dve_uop import DveOpSpec


def _register_op(name, spec):
    for op in _dve_ops.OPS:
        if op.name == name:
            return op
    op = _dve_ops.DveOp(name, spec, subdim=False, uops_sha={})
    _dve_ops.OPS.append(op)
    opcode = _dve_ops._CUSTOM_DVE_ROW_BASE + len(_dve_ops.OPS) - 1
    assert opcode < 0x20
    _dve_ops._SUB_OPCODE_FOR_NAME[name] = opcode
    _dve_ops.CUSTOM_DVE_SPECS[name] = spec
    shas = {}
    for ver in ("v3", "v4"):
        s = DveOpSpec(name=name, opcode=opcode, uops=lower(spec, ver=ver),
                      rd1_en=True)
        shas[ver] = s.sha(ver)
    op = dataclasses.replace(op, uops_sha=shas)
    _dve_ops.OPS[-1] = op
    return op


ADD_MIN_OP = _register_op(
    "ANT_ADD_MIN",
    Spec(
        body=Src0 + Src1,
        accum=AluOp.MIN,
        accum_init=C2,
        reference=lambda in0, in1, s0, s1, imm2: (in0 + in1).astype(np.float32),
    ),
)

ADD_ADD_MAX_OP = _register_op(
    "ANT_ADD_ADD_MAX",
    Spec(
        body=Src0 + Src1 + C2,
        accum=AluOp.MAX,
        accum_init=MaxNeg,
        reference=lambda in0, in1, s0, s1, imm2: (
            in0 + in1 + np.float32(imm2)).astype(np.float32),
    ),
)

B = 4096          # batch (anchors)
D = 512           # embedding dim
N_CORES = 8
ROWS = B // N_CORES      # 512 anchor rows per core
P = 128                  # partitions
MT = ROWS // P           # 4 m-tiles per core
NW = 512                 # psum bank width (fp32)
GW = 2048                # column group width (4 banks)
NG = B // GW             # 2 column groups
KT = D // P              # 4 contraction k-tiles (paired 2x for DoubleRow)

MARGIN = 0.5
EPS = 1e-6
BIG = 65536.0            # accum_init for the min ops
PEN = 8192.0             # band bump baked into the fp16 in1 tiles
BW = 704                 # banded-colterm width (band must fit in [0, BW))
AW = 384                 # band-max window width per m-tile
WLO = [0, 64, 192, 320]  # band-max window start per m-tile (t*128-64, clamped)

_nc_cache = {}


def _build(reps=1):
    nc = bacc.Bacc("TRN2", target_bir_lowering=False)
    fp16 = mybir.dt.float16
    fp8 = mybir.dt.float8e4
    f32 = mybir.dt.float32
    DR = mybir.MatmulPerfMode.DoubleRow

    et = nc.dram_tensor("et", [D, B], fp8, kind="ExternalInput")
    eblk = nc.dram_tensor("eblk", [D, ROWS], fp8, kind="ExternalInput")
    ctb = nc.dram_tensor("ctb", [P, MT * BW], fp16, kind="ExternalInput")
    ctr = nc.dram_tensor("ctr", [P, GW - BW], fp16, kind="ExternalInput")
    ctg1 = nc.dram_tensor("ctg1", [P, GW], fp16, kind="ExternalInput")
    outd = nc.dram_tensor("out", [reps, P, 4 * MT], f32, kind="ExternalOutput")

    with TileContext(nc) as tc:
        with (
            tc.tile_pool(name="etp", bufs=1) as etp,
            tc.tile_pool(name="ebp", bufs=1) as ebp,
            tc.tile_pool(name="wp", bufs=2) as wp,
            tc.tile_pool(name="accp", bufs=2) as accp,
            tc.tile_pool(name="psp", bufs=2, space="PSUM") as psp,
        ):
            # --- PE warmup: dense tiny matmuls while input DMAs run -------
            warm = etp.tile([P, 64], fp16, tag="warm")
            nc.vector.memset(warm, 0.0)
            wps = psp.tile([P, GW], f32, tag="ps", name="wps")
            for _ in range(96):
                nc.tensor.matmul(wps[:64, 0:64], warm[:, 0:64], warm[:, 0:64],
                                 start=True, stop=True)

            # --- input DMAs: ONE queue, in order of first use -------------
            eb_all = ebp.tile([P, KT * ROWS], fp8, tag="eb", name="eb_all")
            et_all = etp.tile([P, KT * B], fp8, tag="et", name="et_all")
            eb4 = eb_all.rearrange("p (k n) -> p k n", k=KT)
            et4 = et_all.rearrange("p (k n) -> p k n", k=KT)
            ebd4 = eblk.rearrange("(k p) n -> p k n", p=P)
            etd4 = et.rearrange("(k p) n -> p k n", p=P)
            ctb_sb = etp.tile([P, MT * BW], fp16, tag="ctb")
            ctr_sb = etp.tile([P, GW - BW], fp16, tag="ctr")
            ctg1_sb = etp.tile([P, GW], fp16, tag="ctg1")

            nc.gpsimd.dma_start(out=eb4, in_=ebd4)
            nc.gpsimd.dma_start(out=et4[:, 0:2, 0:GW], in_=etd4[:, 0:2, 0:GW])
            nc.gpsimd.dma_start(out=et4[:, 2:4, 0:GW], in_=etd4[:, 2:4, 0:GW])
            nc.gpsimd.dma_start(out=ctb_sb[:, 0:BW], in_=ctb[:, 0:BW])
            nc.gpsimd.dma_start(out=ctr_sb, in_=ctr[:, :])
            nc.gpsimd.dma_start(out=et4[:, 0:2, GW:B], in_=etd4[:, 0:2, GW:B])
            nc.gpsimd.dma_start(out=et4[:, 2:4, GW:B], in_=etd4[:, 2:4, GW:B])
            nc.gpsimd.dma_start(out=ctg1_sb, in_=ctg1[:, :])
            nc.gpsimd.dma_start(out=ctb_sb[:, BW:MT * BW],
                                in_=ctb[:, BW:MT * BW])

            for r in range(reps):
                osb = accp.tile([P, 4 * MT], f32, tag="osb", name="osb")
                for t in range(MT):
                    ms = slice(t * P, (t + 1) * P)
                    for g in range(NG):
                        ps = psp.tile([P, GW], f32, tag="ps", name="ps")
                        # k-major: consecutive matmuls share the stationary
                        # operand; gram: w = -2 e_m.e_n (two K=256 passes)
                        for u in range(2):
                            for j in range(GW // NW):
                                cs = slice(g * GW + j * NW,
                                           g * GW + (j + 1) * NW)
                                js = slice(j * NW, (j + 1) * NW)
                                nc.tensor.matmul(
                                    ps[:, js],
                                    eb4[:, 2 * u:2 * u + 2, ms],
                                    et4[:, 2 * u:2 * u + 2, cs],
                                    start=(u == 0), stop=(u == 1),
                                    perf_mode=DR,
                                )
                        scr = wp.tile([P, GW], f32, tag="scr", name="scr")
                        tb = slice(t * BW, (t + 1) * BW)
                        if g == 0:
                            # hardest positive: windowed max of w + ctb - PEN
                            nc.vector._custom_dve(
                                ADD_ADD_MAX_OP,
                                out=scr[:, 0:AW],
                                in0=ps[:, WLO[t]:WLO[t] + AW],
                                in1=ctb_sb[:, t * BW + WLO[t]:
                                           t * BW + WLO[t] + AW],
                                imm2=-PEN,
                                accum_out=osb[:, t:t + 1],
                            )
                            # hardest negative, banded span [0:BW)
                            nc.vector._custom_dve(
                                ADD_MIN_OP,
                                out=scr[:, 0:BW],
                                in0=ps[:, 0:BW],
                                in1=ctb_sb[:, tb],
                                imm2=BIG,
                                accum_out=osb[:, MT + t:MT + t + 1],
                            )
                            # hardest negative, rest of group 0
                            nc.vector._custom_dve(
                                ADD_MIN_OP,
                                out=scr[:, 0:GW - BW],
                                in0=ps[:, BW:GW],
                                in1=ctr_sb,
                                imm2=BIG,
                                accum_out=osb[:, 2 * MT + t:2 * MT + t + 1],
                            )
                        else:
                            # hardest negative, group 1 (no band columns)
                            nc.vector._custom_dve(
                                ADD_MIN_OP,
                                out=scr,
                                in0=ps,
                                in1=ctg1_sb,
                                imm2=BIG,
                                accum_out=osb[:, 3 * MT + t:3 * MT + t + 1],
                            )
                nc.gpsimd.dma_start(out=outd[r], in_=osb)
    nc.compile()
    return nc


def _get_nc(reps=1):
    if reps not in _nc_cache:
        _nc_cache[reps] = _build(reps)
    return _nc_cache[reps]


def _prepare_inputs(embeddings, labels):
    f8 = ml_dtypes.float8_e4m3
    Ef = np.ascontiguousarray(np.asarray(embeddings, dtype=np.float32))
    lab = np.asarray(labels).astype(np.int64)
    perm = np.argsort(lab, kind="stable")
    Ef = Ef[perm]
    labp = lab[perm]

    sq = np.sum(Ef * Ef, axis=1, dtype=np.float32)          # [B]
    s = np.sum(Ef, axis=1, dtype=np.float32)                # [B]
    rowterm = (sq + 2.0 * EPS * s + D * EPS * EPS).astype(np.float32)
    colterm = (sq - 2.0 * EPS * s).astype(np.float32)

    # fp8 embeddings, scaled by sqrt(2) so gram = 2 e.e
    et8 = np.ascontiguousarray(
        (Ef * np.float32(np.sqrt(2.0))).astype(f8).T)          # [D, B]
    en8 = np.ascontiguousarray(
        (Ef * np.float32(-np.sqrt(2.0))).astype(f8).T)         # [D, B]

    seg_start = np.searchsorted(labp, labp, side="left")
    seg_end = np.searchsorted(labp, labp, side="right")

    jj = np.arange(BW)
    in_maps = []
    for c in range(N_CORES):
        r0, r1 = c * ROWS, (c + 1) * ROWS
        w0 = int(seg_start[r0])
        lo_b = (seg_start[r0:r1] - w0).astype(np.int64)
        hi_b = (seg_end[r0:r1] - w0).astype(np.int64)
        colperm = (np.arange(B) + w0) % B
        ctrot = colterm[colperm]

        ctb_a = np.empty((P, MT * BW), dtype=np.float16)
        for t in range(MT):
            tl = lo_b[t * P:(t + 1) * P][:, None]
            th = hi_b[t * P:(t + 1) * P][:, None]
            assert tl.min() >= WLO[t] and th.max() <= WLO[t] + AW, (
                c, t, tl.min(), th.max())
            band = (jj[None, :] >= tl) & (jj[None, :] < th)      # [P, BW]
            ctb_a[:, t * BW:(t + 1) * BW] = (
                ctrot[None, 0:BW] + np.float32(PEN) * band)

        in_maps.append({
            "et": np.ascontiguousarray(et8[:, colperm]),
            "eblk": np.ascontiguousarray(en8[:, r0:r1]),
            "ctb": ctb_a,
            "ctr": np.ascontiguousarray(
                np.broadcast_to(ctrot[None, BW:GW], (P, GW - BW))
            ).astype(np.float16),
            "ctg1": np.ascontiguousarray(
                np.broadcast_to(ctrot[None, GW:B], (P, GW))
            ).astype(np.float16),
        })
    return in_maps, labp, rowterm


def _postprocess(results, labp, rowterm):
    # out[0]: [P, 4*MT]: hp [0:MT], hn parts [MT:2MT], [2MT:3MT], [3MT:4MT]
    hp_l, hn_l = [], []
    for r in results:
        o = r["out"][0]                                   # [P, 4*MT]
        hp_l.append(o[:, 0:MT].T.reshape(-1))
        hn = np.minimum(np.minimum(o[:, MT:2 * MT], o[:, 2 * MT:3 * MT]),
                        o[:, 3 * MT:4 * MT])
        hn_l.append(hn.T.reshape(-1))
    hp_raw = np.concatenate(hp_l)
    hn_raw = np.concatenate(hn_l)
    hp2 = hp_raw + rowterm
    hn2 = hn_raw + rowterm
    hp = np.sqrt(np.maximum(hp2, 0.0, dtype=np.float32))
    hn = np.sqrt(np.maximum(hn2, 0.0, dtype=np.float32))

    cnt_lab = np.bincount(labp, minlength=1)
    n_same = cnt_lab[labp]
    valid = (n_same > 1) & (n_same < B)
    per = np.where(valid, np.maximum(hp - hn + np.float32(MARGIN), 0.0), 0.0)
    cnt = np.float32(valid.sum())
    if cnt > 0:
        loss = np.float32(per.sum(dtype=np.float32) / max(cnt, np.float32(1.0)))
    else:
        loss = np.float32(0.0)
    return np.asarray(loss, dtype=np.float32)


def _run(in_maps, reps=1, **kw):
    nc = _get_nc(reps)
    return run_bass_kernel_spmd(nc, in_maps, core_ids=list(range(N_CORES)), **kw)


def kernel(embeddings, labels):
    in_maps, labp, rowterm = _prepare_inputs(embeddings, labels)
    res = _run(in_maps)
    return _postprocess(res.results, labp, rowterm)
